# revision 6
# baseline (speedup 1.0000x reference)
"""RBF-kernel SVM decision function on 8 TRN2 NeuronCores.

out[i] = sum_j alphas[j] * exp(-GAMMA * ||x[i] - supports[j]||^2)

Strategy (data-parallel over x rows, supports/alphas replicated):
  exponent e_ij = -g|x_i|^2 + (2g x_i . s_j) + (ln|a_j| - g|s_j|^2)
    - 2g x_i.s_j  : bf16 matmul, scaled so PSUM holds e'_ij = (e_ij + g|x_i|^2)
                    * S16 with S16 = 2^10/ln2 (fp16-bit-pattern exponent units)
    - j-term      : folded into the matmul as 2 extra contraction rows (hi/lo
                    bf16 split), scaled by S16
    - i-term      : per-partition bias (ACT bias in natural units, or folded
                    into the int conversion offset on the DVE path)
  out_i = sum_{j: a_j>0} exp(e_ij) - sum_{j: a_j<0} exp(e_ij)
    - supports host-sorted so positive-alpha group comes first (boundary b)

Two consumer paths per j-window (the ScalarE ACTIVATE is the global
bottleneck at 1 elem/cycle/lane, so ~1/4 of windows bypass it):
  ACT window: ACTIVATE(Exp, scale=1/S16, bias=-g|x|^2, accum_out=...) reduces
    in place on PSUM.
  DVE window (fast-exp): two-phase Schraudolph in fp16 bit-pattern:
    t   = round_f32_to_i16(v + cbA)   [cbA = ci*S16 + (15+sigma)*2^10], max 0
    tB  = t + 512  (half octave)
    e^y ~ bitcast_fp16(t)*0.5 + bitcast_fp16(tB)*0.5/sqrt(2)
    (the linear-mantissa error bumps of the two phases nearly cancel:
     residual ~ +-0.75%, which empirically costs ~6e-3 rel per 16 windows)
    reduced by two CACHE_REDUCE ops with the 0.5 / 0.35355 constants as the
    tensor_scalar multiplier; alpha signs via P/N piece splits as usual.
"""

import os
import sys

for p in ("/opt/trn_rl_repo",):
    if p not in sys.path:
        sys.path.insert(0, p)

import numpy as np
import ml_dtypes

import concourse.bass as bass
import concourse.tile as tile
from concourse import bacc, mybir
from concourse.bass_utils import run_bass_kernel_spmd

N_CORES = 8
N = 16384
M = 8192
F = 64
GAMMA = 1.0 / F
N_LOC = N // N_CORES        # 2048 queries per core
N_TILES = N_LOC // 128      # 16 i-tiles of 128 queries
K_AUG = F + 2               # 66 contraction rows
W = 2048                    # j-window: 4 PSUM banks
NW = M // W                 # 4 windows per j sweep
MM_N = 512                  # matmul moving free dim (1 PSUM bank)

LN2 = float(np.log(2.0))
S16 = 2.0**10 / LN2         # exponent pre-scale (fp16 bit-pattern units)
SIGMA = float(os.environ.get("BASS_SIGMA", "-0.0575"))
D_DVE = int(os.environ.get("BASS_D", "18"))  # number of fast-exp windows
CA = 0.5
CB = 0.5 / float(np.sqrt(2.0))

BF16 = mybir.dt.bfloat16
FP16 = mybir.dt.float16
F32 = mybir.dt.float32
I16 = mybir.dt.int16
bf16 = ml_dtypes.bfloat16

_compiled_cache = {}


def _dve_set(d, b):
    """Choose d (tile, window) pairs for the fast-exp path. Prefer windows
    that do not straddle the sign boundary b (halves the reduce count) and
    keep the last tile all-ACT (short tail)."""
    bw = b // W  # window containing the boundary
    order = [w for w in (1, 3, 0, 2) if w != bw] + [bw]
    out = set()
    for w in order:
        for t in range(N_TILES - 1):
            if len(out) >= d:
                return frozenset(out)
            out.add((t, w))
    return frozenset(out)


def _build_common(nc, tc, cpool):
    xaugT_d = nc.dram_tensor("xaugT", [K_AUG, N_LOC], BF16, kind="ExternalInput")
    saug_d = nc.dram_tensor("saug", [K_AUG, M], BF16, kind="ExternalInput")
    cbias_d = nc.dram_tensor("cbias", [128, N_TILES], F32, kind="ExternalInput")
    cbA_d = nc.dram_tensor("cbA", [128, N_TILES], F32, kind="ExternalInput")
    out_d = nc.dram_tensor("out", [128, N_TILES], F32, kind="ExternalOutput")

    # Dummy exp() on a zeroed tile: first in the ACT engine's program, so the
    # ~2.7us exp table load overlaps the input DMAs.
    warm_act = cpool.tile([128, 1], F32)
    nc.gpsimd.memset(warm_act[:], 0.0)
    nc.scalar.activation(warm_act[:], warm_act[:], mybir.ActivationFunctionType.Exp)

    saug_sb = cpool.tile([K_AUG, M], BF16)
    nc.sync.dma_start(saug_sb[:, 0:W], saug_d.ap()[:, 0:W])
    xaugT_sb = cpool.tile([K_AUG, N_LOC], BF16)
    nc.sync.dma_start(xaugT_sb[:, 0:128], xaugT_d.ap()[:, 0:128])
    cbias_sb = cpool.tile([128, N_TILES], F32)
    nc.sync.dma_start(cbias_sb[:], cbias_d.ap()[:])
    cbA_sb = cpool.tile([128, N_TILES], F32)
    nc.sync.dma_start(cbA_sb[:], cbA_d.ap()[:])
    for w in range(1, NW):
        nc.sync.dma_start(
            saug_sb[:, w * W : (w + 1) * W],
            saug_d.ap()[:, w * W : (w + 1) * W],
        )
    nc.sync.dma_start(xaugT_sb[:, 128:], xaugT_d.ap()[:, 128:])
    return xaugT_sb, saug_sb, cbias_sb, cbA_sb, out_d


def _mm_windows(nc, t, ps_tile, w, xaugT_sb, saug_sb):
    for c in range(W // MM_N):
        nc.tensor.matmul(
            ps_tile[:, c * MM_N : (c + 1) * MM_N],
            xaugT_sb[:, t * 128 : (t + 1) * 128],
            saug_sb[:, w * W + c * MM_N : w * W + (c + 1) * MM_N],
            start=True,
            stop=True,
        )


def _build_v4(b, dve_ws):
    nc = bacc.Bacc(
        "TRN2",
        target_bir_lowering=False,
        debug=False,
        enable_asserts=False,
        num_devices=N_CORES,
    )

    def pieces_of(w):
        lo, hi = w * W, (w + 1) * W
        if b <= lo:
            return [(lo, hi, False)]
        if b >= hi:
            return [(lo, hi, True)]
        return [(lo, b, True), (b, hi, False)]

    def tile_counts(t):
        nP = nN = 0
        for w in range(NW):
            mult = 2 if (t, w) in dve_ws else 1
            for _, _, pos in pieces_of(w):
                if pos:
                    nP += mult
                else:
                    nN += mult
        return nP, nN

    with tile.TileContext(nc) as tc:
        with (
            tc.tile_pool(name="const", bufs=1) as cpool,
            tc.tile_pool(name="acc", bufs=6) as apool,
            tc.tile_pool(name="stg", bufs=3) as spool,
            tc.tile_pool(name="psum", bufs=2, space="PSUM") as ppool,
        ):
            xaugT_sb, saug_sb, cbias_sb, cbA_sb, out_d = _build_common(nc, tc, cpool)
            outT_sb = cpool.tile([128, N_TILES], F32)
            dvout = cpool.tile([128, 2 * W], FP16)
            pending = []  # deferred per-tile finishers (decouple DVE FIFO from ACT)

            for t in range(N_TILES):
                nP, nN = tile_counts(t)
                accP = apool.tile([128, max(nP, 1)], F32, tag="accP")
                accN = apool.tile([128, max(nN, 1)], F32, tag="accN")
                iP = iN = 0

                def acc_col(pos):
                    nonlocal iP, iN
                    if pos:
                        col = accP[:, iP : iP + 1]
                        iP += 1
                    else:
                        col = accN[:, iN : iN + 1]
                        iN += 1
                    return col

                for w in range(NW):
                    ps_tile = ppool.tile([128, W], F32, tag="E")
                    _mm_windows(nc, t, ps_tile, w, xaugT_sb, saug_sb)
                    if (t, w) in dve_ws:
                        stg = spool.tile([128, 2 * W], FP16, tag="stg")
                        nc.vector.tensor_scalar(
                            stg[:, 0:W].bitcast(I16),
                            ps_tile[:],
                            cbA_sb[:, t : t + 1],
                            0.0,
                            mybir.AluOpType.add,
                            mybir.AluOpType.max,
                        )
                        nc.vector.tensor_scalar(
                            stg[:, W : 2 * W].bitcast(I16),
                            stg[:, 0:W].bitcast(I16),
                            512.0,
                            None,
                            mybir.AluOpType.add,
                        )
                        for off, cph in ((0, CA), (W, CB)):
                            for lo, hi, pos in pieces_of(w):
                                nc.vector.tensor_scalar(
                                    dvout[:, off + lo - w * W : off + hi - w * W],
                                    stg[:, off + lo - w * W : off + hi - w * W],
                                    cph,
                                    0.0,
                                    mybir.AluOpType.mult,
                                    mybir.AluOpType.add,
                                    accum_out=acc_col(pos),
                                )
                    else:
                        for lo, hi, pos in pieces_of(w):
                            nc.scalar.activation(
                                ps_tile[:, lo - w * W : hi - w * W],
                                ps_tile[:, lo - w * W : hi - w * W],
                                mybir.ActivationFunctionType.Exp,
                                bias=cbias_sb[:, t : t + 1],
                                scale=1.0 / S16,
                                accum_out=acc_col(pos),
                            )
                def finisher(t=t, accP=accP, accN=accN, iP=iP, iN=iN):
                    sumP = apool.tile([128, 1], F32, tag="sumP")
                    nc.vector.reduce_sum(
                        sumP[:], accP[:, :iP], axis=mybir.AxisListType.X
                    )
                    sumN = apool.tile([128, 1], F32, tag="sumN")
                    nc.vector.reduce_sum(
                        sumN[:], accN[:, :iN], axis=mybir.AxisListType.X
                    )
                    nc.vector.tensor_sub(outT_sb[:, t : t + 1], sumP[:], sumN[:])

                pending.append(finisher)
                if len(pending) > 2:
                    pending.pop(0)()

            for fin in pending:
                fin()

            nc.sync.dma_start(out_d.ap()[:], outT_sb[:])

    nc.compile()
    return nc


def _prepare(x, supports, alphas):
    x = np.asarray(x, dtype=np.float32)
    supports = np.asarray(supports, dtype=np.float32)
    alphas = np.asarray(alphas, dtype=np.float32)

    a64 = alphas.astype(np.float64)
    s64 = supports.astype(np.float64)
    jterm = -GAMMA * (s64 * s64).sum(axis=1) + np.log(
        np.maximum(np.abs(a64), 1e-300)
    )
    jterm = np.maximum(jterm, -34.0)

    pos = a64 > 0
    perm = np.concatenate([np.nonzero(pos)[0], np.nonzero(~pos)[0]])
    b = int(pos.sum())

    jt = jterm[perm] * S16
    hi = jt.astype(bf16)
    lo = (jt - hi.astype(np.float64)).astype(bf16)

    saug = np.empty((K_AUG, M), dtype=bf16)
    saug[:F] = (supports[perm].T.astype(np.float64) * (2.0 * GAMMA * S16 * 32.0)).astype(bf16)
    saug[F] = hi
    saug[F + 1] = lo

    xaugT = np.ones((K_AUG, N), dtype=bf16)
    xaugT[:F] = (x.T / 32.0).astype(bf16)

    ci = (-GAMMA * (x.astype(np.float64) ** 2).sum(axis=1))
    cbias = ci.astype(np.float32)
    cbA = (ci * S16 + (15.0 + SIGMA) * 2.0**10).astype(np.float32)

    in_maps = []
    for c in range(N_CORES):
        sl = slice(c * N_LOC, (c + 1) * N_LOC)
        in_maps.append(
            {
                "xaugT": np.ascontiguousarray(xaugT[:, sl]),
                "saug": saug,
                "cbias": np.ascontiguousarray(
                    cbias[sl].reshape(N_TILES, 128).T
                ),
                "cbA": np.ascontiguousarray(
                    cbA[sl].reshape(N_TILES, 128).T
                ),
            }
        )
    return b, in_maps


def _run(x, supports, alphas, trace=False, **run_kwargs):
    b, in_maps = _prepare(x, supports, alphas)
    dve_ws = _dve_set(D_DVE, b)
    key = (b, dve_ws, SIGMA)
    if key not in _compiled_cache:
        _compiled_cache[key] = _build_v4(b, dve_ws)
    nc = _compiled_cache[key]
    res = run_bass_kernel_spmd(
        nc, in_maps, core_ids=list(range(N_CORES)), trace=trace, **run_kwargs
    )
    outs = [r["out"].T.reshape(-1) for r in res.results]
    return np.concatenate(outs).astype(np.float32), res


def kernel(x, supports, alphas):
    out, _ = _run(x, supports, alphas, trace=False)
    return out


# revision 9
# speedup vs baseline: 1.1139x; 1.1139x over previous
"""RBF-kernel SVM decision function on 8 TRN2 NeuronCores.

out[i] = sum_j alphas[j] * exp(-GAMMA * ||x[i] - supports[j]||^2)

Strategy (data-parallel over x rows, supports/alphas replicated):
  exponent e_ij = -g|x_i|^2 + (2g x_i . s_j) + (ln|a_j| - g|s_j|^2)
    - 2g x_i.s_j  : bf16 matmul, scaled so PSUM holds e'_ij = (e_ij + g|x_i|^2)
                    * S16 with S16 = 2^10/ln2 (fp16-bit-pattern exponent units)
    - j-term      : folded into the matmul as 2 extra contraction rows (hi/lo
                    bf16 split), scaled by S16
    - i-term      : per-partition bias (ACT bias in natural units, or folded
                    into the int conversion offset on the DVE path)
  out_i = sum_{j: a_j>0} exp(e_ij) - sum_{j: a_j<0} exp(e_ij)
    - supports host-sorted so positive-alpha group comes first (boundary b)

Two consumer paths per j-window (the ScalarE ACTIVATE is the global
bottleneck at 1 elem/cycle/lane, so ~1/4 of windows bypass it):
  ACT window: ACTIVATE(Exp, scale=1/S16, bias=-g|x|^2, accum_out=...) reduces
    in place on PSUM.
  DVE window (fast-exp): two-phase Schraudolph in fp16 bit-pattern:
    t   = round_f32_to_i16(v + cbA)   [cbA = ci*S16 + (15+sigma)*2^10], max 0
    tB  = t + 512  (half octave)
    e^y ~ bitcast_fp16(t)*0.5 + bitcast_fp16(tB)*0.5/sqrt(2)
    (the linear-mantissa error bumps of the two phases nearly cancel:
     residual ~ +-0.75%, which empirically costs ~6e-3 rel per 16 windows)
    reduced by two CACHE_REDUCE ops with the 0.5 / 0.35355 constants as the
    tensor_scalar multiplier; alpha signs via P/N piece splits as usual.
"""

import os
import sys

for p in ("/opt/trn_rl_repo",):
    if p not in sys.path:
        sys.path.insert(0, p)

import numpy as np
import ml_dtypes

import concourse.bass as bass
import concourse.tile as tile
from concourse import bacc, mybir
from concourse.bass_utils import run_bass_kernel_spmd

N_CORES = 8
N = 16384
M = 8192
F = 64
GAMMA = 1.0 / F
N_LOC = N // N_CORES        # 2048 queries per core
N_TILES = N_LOC // 128      # 16 i-tiles of 128 queries
K_AUG = F + 2               # 66 contraction rows
W = 2048                    # j-window: 4 PSUM banks
NW = M // W                 # 4 windows per j sweep
MM_N = 512                  # matmul moving free dim (1 PSUM bank)

LN2 = float(np.log(2.0))
S16 = 2.0**10 / LN2         # exponent pre-scale (fp16 bit-pattern units)
SIGMA = float(os.environ.get("BASS_SIGMA", "-0.0575"))
D_DVE = int(os.environ.get("BASS_D", "15"))  # number of fast-exp windows
CA = 0.5
CB = 0.5 / float(np.sqrt(2.0))

BF16 = mybir.dt.bfloat16
FP16 = mybir.dt.float16
F32 = mybir.dt.float32
I16 = mybir.dt.int16
bf16 = ml_dtypes.bfloat16

_compiled_cache = {}


def _dve_set(d, b):
    """Choose d (tile, window) pairs for the fast-exp path: at most one per
    tile (keeps per-tile ACT/DVE load even), never the window straddling the
    sign boundary b, and keep the last tile all-ACT (short tail)."""
    bw = b // W  # window containing the boundary
    w_pick = next(w for w in (1, 3, 0, 2) if w != bw)
    out = set()
    for t in range(min(d, N_TILES - 1)):
        out.add((t, w_pick))
    return frozenset(out)


def _build_common(nc, tc, cpool):
    xaugT_d = nc.dram_tensor("xaugT", [K_AUG, N_LOC], BF16, kind="ExternalInput")
    saug_d = nc.dram_tensor("saug", [K_AUG, M], BF16, kind="ExternalInput")
    cbias_d = nc.dram_tensor("cbias", [128, N_TILES], F32, kind="ExternalInput")
    cbA_d = nc.dram_tensor("cbA", [128, N_TILES], F32, kind="ExternalInput")
    out_d = nc.dram_tensor("out", [128, N_TILES], F32, kind="ExternalOutput")

    # Dummy exp() on a zeroed tile: first in the ACT engine's program, so the
    # ~2.7us exp table load overlaps the input DMAs.
    warm_act = cpool.tile([128, 1], F32)
    nc.gpsimd.memset(warm_act[:], 0.0)
    nc.scalar.activation(warm_act[:], warm_act[:], mybir.ActivationFunctionType.Exp)

    saug_sb = cpool.tile([K_AUG, M], BF16)
    nc.sync.dma_start(saug_sb[:, 0:W], saug_d.ap()[:, 0:W])
    xaugT_sb = cpool.tile([K_AUG, N_LOC], BF16)
    nc.sync.dma_start(xaugT_sb[:, 0:128], xaugT_d.ap()[:, 0:128])
    cbias_sb = cpool.tile([128, N_TILES], F32)
    nc.sync.dma_start(cbias_sb[:], cbias_d.ap()[:])
    cbA_sb = cpool.tile([128, N_TILES], F32)
    nc.sync.dma_start(cbA_sb[:], cbA_d.ap()[:])
    for w in range(1, NW):
        nc.sync.dma_start(
            saug_sb[:, w * W : (w + 1) * W],
            saug_d.ap()[:, w * W : (w + 1) * W],
        )
    nc.sync.dma_start(xaugT_sb[:, 128:], xaugT_d.ap()[:, 128:])
    return xaugT_sb, saug_sb, cbias_sb, cbA_sb, out_d


def _mm_windows(nc, t, ps_tile, w, xaugT_sb, saug_sb):
    for c in range(W // MM_N):
        nc.tensor.matmul(
            ps_tile[:, c * MM_N : (c + 1) * MM_N],
            xaugT_sb[:, t * 128 : (t + 1) * 128],
            saug_sb[:, w * W + c * MM_N : w * W + (c + 1) * MM_N],
            start=True,
            stop=True,
        )


def _build_v4(b, dve_ws):
    nc = bacc.Bacc(
        "TRN2",
        target_bir_lowering=False,
        debug=False,
        enable_asserts=False,
        num_devices=N_CORES,
    )

    def pieces_of(w):
        lo, hi = w * W, (w + 1) * W
        if b <= lo:
            return [(lo, hi, False)]
        if b >= hi:
            return [(lo, hi, True)]
        return [(lo, b, True), (b, hi, False)]

    def tile_counts(t):
        nP = nN = 0
        for w in range(NW):
            mult = 2 if (t, w) in dve_ws else 1
            for _, _, pos in pieces_of(w):
                if pos:
                    nP += mult
                else:
                    nN += mult
        return nP, nN

    with tile.TileContext(nc) as tc:
        with (
            tc.tile_pool(name="const", bufs=1) as cpool,
            tc.tile_pool(name="acc", bufs=6) as apool,
            tc.tile_pool(name="stg", bufs=3) as spool,
            tc.tile_pool(name="psum", bufs=2, space="PSUM") as ppool,
        ):
            xaugT_sb, saug_sb, cbias_sb, cbA_sb, out_d = _build_common(nc, tc, cpool)
            outT_sb = cpool.tile([128, N_TILES], F32)
            dvout = cpool.tile([128, 2 * W], FP16)
            pending = []  # deferred per-tile finishers (decouple DVE FIFO from ACT)

            for t in range(N_TILES):
                nP, nN = tile_counts(t)
                accP = apool.tile([128, max(nP, 1)], F32, tag="accP")
                accN = apool.tile([128, max(nN, 1)], F32, tag="accN")
                iP = iN = 0

                def acc_col(pos):
                    nonlocal iP, iN
                    if pos:
                        col = accP[:, iP : iP + 1]
                        iP += 1
                    else:
                        col = accN[:, iN : iN + 1]
                        iN += 1
                    return col

                for w in range(NW):
                    ps_tile = ppool.tile([128, W], F32, tag="E")
                    _mm_windows(nc, t, ps_tile, w, xaugT_sb, saug_sb)
                    if (t, w) in dve_ws:
                        stg = spool.tile([128, 2 * W], FP16, tag="stg")
                        nc.vector.tensor_scalar(
                            stg[:, 0:W].bitcast(I16),
                            ps_tile[:],
                            cbA_sb[:, t : t + 1],
                            0.0,
                            mybir.AluOpType.add,
                            mybir.AluOpType.max,
                        )
                        nc.vector.tensor_scalar(
                            stg[:, W : 2 * W].bitcast(I16),
                            stg[:, 0:W].bitcast(I16),
                            512.0,
                            None,
                            mybir.AluOpType.add,
                        )
                        (_, _, pos), = pieces_of(w)  # never straddled
                        # per phase: 2-level pairwise tree (fp16 TT at 2x)
                        # then a 512-wide CACHE_REDUCE with the phase const
                        h1 = spool.tile([128, W], FP16, tag="h1")
                        h2 = spool.tile([128, W // 2], FP16, tag="h2")
                        for k, cph in ((0, CA), (1, CB)):
                            o = k * W
                            t1 = h1[:, k * (W // 2) : (k + 1) * (W // 2)]
                            nc.vector.tensor_tensor(
                                t1,
                                stg[:, o : o + W // 2],
                                stg[:, o + W // 2 : o + W],
                                mybir.AluOpType.add,
                            )
                            t2 = h2[:, k * (W // 4) : (k + 1) * (W // 4)]
                            nc.vector.tensor_tensor(
                                t2,
                                t1[:, 0 : W // 4],
                                t1[:, W // 4 : W // 2],
                                mybir.AluOpType.add,
                            )
                            nc.vector.tensor_scalar(
                                dvout[:, k * (W // 4) : (k + 1) * (W // 4)],
                                t2,
                                cph,
                                0.0,
                                mybir.AluOpType.mult,
                                mybir.AluOpType.add,
                                accum_out=acc_col(pos),
                            )
                    else:
                        for lo, hi, pos in pieces_of(w):
                            nc.scalar.activation(
                                ps_tile[:, lo - w * W : hi - w * W],
                                ps_tile[:, lo - w * W : hi - w * W],
                                mybir.ActivationFunctionType.Exp,
                                bias=cbias_sb[:, t : t + 1],
                                scale=1.0 / S16,
                                accum_out=acc_col(pos),
                            )
                def finisher(t=t, accP=accP, accN=accN, iP=iP, iN=iN):
                    sumP = apool.tile([128, 1], F32, tag="sumP")
                    nc.vector.reduce_sum(
                        sumP[:], accP[:, :iP], axis=mybir.AxisListType.X
                    )
                    sumN = apool.tile([128, 1], F32, tag="sumN")
                    nc.vector.reduce_sum(
                        sumN[:], accN[:, :iN], axis=mybir.AxisListType.X
                    )
                    nc.vector.tensor_sub(outT_sb[:, t : t + 1], sumP[:], sumN[:])

                pending.append(finisher)
                if len(pending) > 2:
                    pending.pop(0)()

            for fin in pending:
                fin()

            nc.sync.dma_start(out_d.ap()[:], outT_sb[:])

    nc.compile()
    return nc


def _prepare(x, supports, alphas):
    x = np.asarray(x, dtype=np.float32)
    supports = np.asarray(supports, dtype=np.float32)
    alphas = np.asarray(alphas, dtype=np.float32)

    a64 = alphas.astype(np.float64)
    s64 = supports.astype(np.float64)
    jterm = -GAMMA * (s64 * s64).sum(axis=1) + np.log(
        np.maximum(np.abs(a64), 1e-300)
    )
    jterm = np.maximum(jterm, -34.0)

    pos = a64 > 0
    perm = np.concatenate([np.nonzero(pos)[0], np.nonzero(~pos)[0]])
    b = int(pos.sum())

    jt = jterm[perm] * S16
    hi = jt.astype(bf16)
    lo = (jt - hi.astype(np.float64)).astype(bf16)

    saug = np.empty((K_AUG, M), dtype=bf16)
    saug[:F] = (supports[perm].T.astype(np.float64) * (2.0 * GAMMA * S16 * 32.0)).astype(bf16)
    saug[F] = hi
    saug[F + 1] = lo

    xaugT = np.ones((K_AUG, N), dtype=bf16)
    xaugT[:F] = (x.T / 32.0).astype(bf16)

    ci = (-GAMMA * (x.astype(np.float64) ** 2).sum(axis=1))
    cbias = ci.astype(np.float32)
    cbA = (ci * S16 + (15.0 + SIGMA) * 2.0**10).astype(np.float32)

    in_maps = []
    for c in range(N_CORES):
        sl = slice(c * N_LOC, (c + 1) * N_LOC)
        in_maps.append(
            {
                "xaugT": np.ascontiguousarray(xaugT[:, sl]),
                "saug": saug,
                "cbias": np.ascontiguousarray(
                    cbias[sl].reshape(N_TILES, 128).T
                ),
                "cbA": np.ascontiguousarray(
                    cbA[sl].reshape(N_TILES, 128).T
                ),
            }
        )
    return b, in_maps


def _run(x, supports, alphas, trace=False, **run_kwargs):
    b, in_maps = _prepare(x, supports, alphas)
    dve_ws = _dve_set(D_DVE, b)
    key = (b, dve_ws, SIGMA)
    if key not in _compiled_cache:
        _compiled_cache[key] = _build_v4(b, dve_ws)
    nc = _compiled_cache[key]
    res = run_bass_kernel_spmd(
        nc, in_maps, core_ids=list(range(N_CORES)), trace=trace, **run_kwargs
    )
    outs = [r["out"].T.reshape(-1) for r in res.results]
    return np.concatenate(outs).astype(np.float32), res


def kernel(x, supports, alphas):
    out, _ = _run(x, supports, alphas, trace=False)
    return out


# revision 11
# speedup vs baseline: 1.2042x; 1.0811x over previous
"""RBF-kernel SVM decision function on 8 TRN2 NeuronCores (baseline v1).

out[i] = sum_j alphas[j] * exp(-GAMMA * ||x[i] - supports[j]||^2)

Hybrid ACT/DVE reduction; see kernel.py history. Known HW exec: ~146.9us.
"""

import os
import sys

for p in ("/opt/trn_rl_repo",):
    if p not in sys.path:
        sys.path.insert(0, p)

import numpy as np
import ml_dtypes

import concourse.bass as bass
import concourse.tile as tile
from concourse import bacc, mybir
from concourse.bass_utils import run_bass_kernel_spmd

N_CORES = 8
N = 16384
M = 8192
F = 64
GAMMA = 1.0 / F
N_LOC = N // N_CORES        # 2048 queries per core
N_TILES = N_LOC // 128      # 16 i-tiles of 128 queries
K_AUG = F + 2               # 66 contraction rows
W = 2048                    # j-window: 4 PSUM banks
NW = M // W                 # 4 windows per j sweep
MM_N = 512                  # matmul moving free dim (1 PSUM bank)
M_PAD = M + 256             # fp16 staging width (zero tail pad, mult of 4)

BF16 = mybir.dt.bfloat16
FP16 = mybir.dt.float16
F32 = mybir.dt.float32
bf16 = ml_dtypes.bfloat16

_compiled_cache = {}


def _build_common(nc, tc, cpool):
    xaugT_d = nc.dram_tensor("xaugT", [K_AUG, N_LOC], BF16, kind="ExternalInput")
    saug_d = nc.dram_tensor("saug", [K_AUG, M], BF16, kind="ExternalInput")
    cbias_d = nc.dram_tensor("cbias", [128, N_TILES], F32, kind="ExternalInput")
    out_d = nc.dram_tensor("out", [128, N_TILES], F32, kind="ExternalOutput")

    warm_act = cpool.tile([128, 1], F32)
    nc.gpsimd.memset(warm_act[:], 0.0)
    nc.scalar.activation(warm_act[:], warm_act[:], mybir.ActivationFunctionType.Exp)

    saug_sb = cpool.tile([K_AUG, M], BF16)
    nc.sync.dma_start(saug_sb[:, 0:W], saug_d.ap()[:, 0:W])
    xaugT_sb = cpool.tile([K_AUG, N_LOC], BF16)
    nc.sync.dma_start(xaugT_sb[:, 0:128], xaugT_d.ap()[:, 0:128])
    cbias_sb = cpool.tile([128, N_TILES], F32)
    nc.sync.dma_start(cbias_sb[:], cbias_d.ap()[:])
    for w in range(1, NW):
        nc.sync.dma_start(
            saug_sb[:, w * W : (w + 1) * W],
            saug_d.ap()[:, w * W : (w + 1) * W],
        )
    nc.sync.dma_start(xaugT_sb[:, 128:], xaugT_d.ap()[:, 128:])
    return xaugT_sb, saug_sb, cbias_sb, out_d


def _mm_windows(nc, t, ps_tile, w, xaugT_sb, saug_sb):
    for c in range(W // MM_N):
        nc.tensor.matmul(
            ps_tile[:, c * MM_N : (c + 1) * MM_N],
            xaugT_sb[:, t * 128 : (t + 1) * 128],
            saug_sb[:, w * W + c * MM_N : w * W + (c + 1) * MM_N],
            start=True,
            stop=True,
        )


def _build_dve_accum(b):
    nc = bacc.Bacc(
        "TRN2",
        target_bir_lowering=False,
        debug=False,
        enable_asserts=False,
        num_devices=N_CORES,
    )
    w_mix = b // W
    act_w = 0 if w_mix != 0 else 1
    dve_ws = [w for w in range(NW) if w != act_w]

    def pieces_of(w):
        lo, hi = w * W, (w + 1) * W
        if b <= lo:
            return [(lo, hi, False)]
        if b >= hi:
            return [(lo, hi, True)]
        return [(lo, b, True), (b, hi, False)]

    n_pos = sum(1 for w in range(NW) for p in pieces_of(w) if p[2])
    n_neg = sum(1 for w in range(NW) for p in pieces_of(w) if not p[2])

    with tile.TileContext(nc) as tc:
        with (
            tc.tile_pool(name="const", bufs=1) as cpool,
            tc.tile_pool(name="acc", bufs=3) as apool,
            tc.tile_pool(name="stg", bufs=3) as spool,
            tc.tile_pool(name="psum", bufs=2, space="PSUM") as ppool,
        ):
            xaugT_sb, saug_sb, cbias_sb, out_d = _build_common(nc, tc, cpool)
            outT_sb = cpool.tile([128, N_TILES], F32)
            dvout = cpool.tile([128, M], FP16)

            for t in range(N_TILES):
                accP = apool.tile([128, max(n_pos, 1)], F32, tag="accP")
                accN = apool.tile([128, max(n_neg, 1)], F32, tag="accN")
                iP = iN = 0

                def acc_col(pos):
                    nonlocal iP, iN
                    if pos:
                        col = accP[:, iP : iP + 1]
                        iP += 1
                    else:
                        col = accN[:, iN : iN + 1]
                        iN += 1
                    return col

                last = t == N_TILES - 1
                if last:
                    act_set = set(range(NW))
                elif t % 2 == 0:
                    act_set = set()
                else:
                    act_set = {act_w}
                stg = spool.tile([128, M], FP16, tag="stg")
                for w in range(NW):
                    ps_tile = ppool.tile([128, W], F32, tag="E")
                    _mm_windows(nc, t, ps_tile, w, xaugT_sb, saug_sb)
                    if w in act_set:
                        for lo, hi, pos in pieces_of(w):
                            nc.scalar.activation(
                                ps_tile[:, lo - w * W : hi - w * W],
                                ps_tile[:, lo - w * W : hi - w * W],
                                mybir.ActivationFunctionType.Exp,
                                bias=cbias_sb[:, t : t + 1],
                                accum_out=acc_col(pos),
                            )
                    else:
                        nc.scalar.activation(
                            stg[:, w * W : (w + 1) * W],
                            ps_tile[:],
                            mybir.ActivationFunctionType.Exp,
                            bias=cbias_sb[:, t : t + 1],
                        )
                for w in range(NW):
                    if w in act_set:
                        continue
                    for lo, hi, pos in pieces_of(w):
                        nc.vector.tensor_scalar(
                            dvout[:, lo:hi],
                            stg[:, lo:hi],
                            1.0,
                            0.0,
                            mybir.AluOpType.mult,
                            mybir.AluOpType.add,
                            accum_out=acc_col(pos),
                        )
                sumP = apool.tile([128, 1], F32, tag="sumP")
                nc.vector.reduce_sum(sumP[:], accP[:, :iP], axis=mybir.AxisListType.X)
                sumN = apool.tile([128, 1], F32, tag="sumN")
                nc.vector.reduce_sum(sumN[:], accN[:, :iN], axis=mybir.AxisListType.X)
                nc.vector.tensor_sub(outT_sb[:, t : t + 1], sumP[:], sumN[:])

            nc.sync.dma_start(out_d.ap()[:], outT_sb[:])

    nc.compile()
    return nc


def _prepare(x, supports, alphas):
    x = np.asarray(x, dtype=np.float32)
    supports = np.asarray(supports, dtype=np.float32)
    alphas = np.asarray(alphas, dtype=np.float32)

    a64 = alphas.astype(np.float64)
    s64 = supports.astype(np.float64)
    jterm = -GAMMA * (s64 * s64).sum(axis=1) + np.log(
        np.maximum(np.abs(a64), 1e-300)
    )

    pos = a64 > 0
    perm = np.concatenate([np.nonzero(pos)[0], np.nonzero(~pos)[0]])
    b = int(pos.sum())

    jt = jterm[perm]
    hi = jt.astype(bf16)
    lo = (jt - hi.astype(np.float64)).astype(bf16)

    saug = np.empty((K_AUG, M), dtype=bf16)
    saug[:F] = supports[perm].T.astype(bf16)
    saug[F] = hi
    saug[F + 1] = lo

    xaugT = np.ones((K_AUG, N), dtype=bf16)
    xaugT[:F] = (x.T / 32.0).astype(bf16)

    cbias = (-GAMMA * (x.astype(np.float64) ** 2).sum(axis=1)).astype(np.float32)

    in_maps = []
    for c in range(N_CORES):
        sl = slice(c * N_LOC, (c + 1) * N_LOC)
        in_maps.append(
            {
                "xaugT": np.ascontiguousarray(xaugT[:, sl]),
                "saug": saug,
                "cbias": np.ascontiguousarray(
                    cbias[sl].reshape(N_TILES, 128).T
                ),
            }
        )
    return b, in_maps


def _run(x, supports, alphas, trace=False, **run_kwargs):
    b, in_maps = _prepare(x, supports, alphas)
    key = (b, "v1")
    if key not in _compiled_cache:
        _compiled_cache[key] = _build_dve_accum(b)
    nc = _compiled_cache[key]
    res = run_bass_kernel_spmd(
        nc, in_maps, core_ids=list(range(N_CORES)), trace=trace, **run_kwargs
    )
    outs = [r["out"].T.reshape(-1) for r in res.results]
    return np.concatenate(outs).astype(np.float32), res


def kernel(x, supports, alphas):
    out, _ = _run(x, supports, alphas, trace=False)
    return out
